# revision 14
# baseline (speedup 1.0000x reference)
"""AlphaDock MDN head kernel for Trainium2 (Bass/Tile), 8-core data-parallel.

Sharding (per spec hint): batch B=8 across the 8 NeuronCores, one graph per
core.  Each core computes its own [NL, NT] pair block end-to-end:
  z = BN(hl@W1a + ht@W1b + b1)   [BN folded into the weights on host]
  C = elu(z); pi/sigma/mu MDN heads; pairwise distances; atom/bond types.

Kernel structure highlights:
  * elu(x)+1 = max(x,0) + min(exp(x),1) (exact), and exp(zl+zt) =
    exp(zl)*exp(zt), so the 4.2M-element exp collapses to two small exps
    plus one fused DVE tensor_scalar (mult+min) per row-chunk at 2x rate.
  * E + R is never materialized: the head matmul accumulates
    b30' + E@W30 + R@W30 in PSUM (a rank-1 ones-matmul adds the bias;
    b30' = b30 - colsum(W30) folds elu's "-1").  relu(z) rows alternate
    between ACT (fused per-partition bias) and GPSIMD to balance engines.
  * pairwise dist: one K=5 matmul gives d2 = |pl|^2 - 2 pl.pt + |pt|^2
    directly in PSUM; sqrt(d2) = exp(0.5*ln(d2)) keeps everything in the
    natural_log_exp activation-table set (zero table switches).
  * pi softmax skips max-subtraction (logits are BN-bounded and small).
  * Outputs interleave as [pairs, 32] f32 rows (pi|sigma|mu|dist|pad) so
    every output-DMA chunk is 128B-contiguous in DRAM; the host splits
    columns when unsharding.
"""

import os

import numpy as np

B, NL, NT = 8, 64, 512
CF = 128
H = 128
K = 10
E = 1024
BN_EPS = 1e-4

NCORES = 8
EC = E // NCORES          # 128 edges per core
PAIRS = NL * NT           # 32768 pairs per core
G = 8                     # l-rows per super-chunk
NSUP = NL // G
BLK = 4 * G               # 128-pair blocks per super-chunk (32)
POUT_BANKS = (BLK * 32 + 511) // 512
_POUTB = 2

# per-l-row engine for relu(z): 'a' = ACT (fused bias), 'g' = GPSIMD.
# Alternating halves the per-engine load; both feed the R-matmul.
_PAT = os.environ.get("KPAT", "gagagaga")
_TT_GPS = os.environ.get("KTT", "dve") == "gps"
_BIGB = int(os.environ.get("KBIGB", "2"))
_HDB = int(os.environ.get("KHDB", "2"))
_TRB = int(os.environ.get("KTRB", "2"))

# packed weight layout (columns in wpack [128, 327])
_W1A0, _W1B0, _W300, _WAT0, _WBS0, _WBD0, _CV0 = 0, 128, 256, 286, 314, 320, 326
_WPACKN = 327
# packed row-vector layout (wrow [1, 546])
_B300, _BAT0, _BBD0 = 0, 512, 540
_WROWN = 546

_PROGRAM = None


def build_program():
    import concourse.bass as bass
    import concourse.bacc as bacc
    import concourse.tile as tile
    from concourse import mybir
    from concourse.masks import make_identity

    f32 = mybir.dt.float32
    i32 = mybir.dt.int32
    AF = mybir.ActivationFunctionType
    OP = mybir.AluOpType
    AX = mybir.AxisListType

    def bap(t, off_extra, dims):
        return bass.AP(t.tensor, t.offset + off_extra, dims)

    nc = bacc.Bacc(None, target_bir_lowering=False)

    # ---------------- DRAM I/O ----------------
    d_hl = nc.dram_tensor("hl", [NL, CF], f32, kind="ExternalInput")
    d_ht = nc.dram_tensor("ht", [NT, CF], f32, kind="ExternalInput")
    d_hlx = nc.dram_tensor("hlx", [B * NL, CF], f32, kind="ExternalInput")
    d_pl = nc.dram_tensor("pl", [NL, 3], f32, kind="ExternalInput")
    d_pt = nc.dram_tensor("pt", [NT, 3], f32, kind="ExternalInput")
    d_es = nc.dram_tensor("esrc", [EC, 1], i32, kind="ExternalInput")
    d_ed = nc.dram_tensor("edst", [EC, 1], i32, kind="ExternalInput")
    d_wp = nc.dram_tensor("wpack", [CF, _WPACKN], f32, kind="ExternalInput")
    d_wr = nc.dram_tensor("wrow", [1, _WROWN], f32, kind="ExternalInput")
    bf16 = mybir.dt.bfloat16
    d_bhl = nc.dram_tensor("b30hl", [2, 512], bf16, kind="ExternalInput")
    d_cbv = nc.dram_tensor("cbval", [1, 1], i32, kind="ExternalInput")

    d_out = nc.dram_tensor("out_all", [PAIRS, 32], f32, kind="ExternalOutput")
    d_atom = nc.dram_tensor("atom_o", [NL, 28], f32, kind="ExternalOutput")
    d_bond = nc.dram_tensor("bond_o", [EC, 6], f32, kind="ExternalOutput")
    d_cb = nc.dram_tensor("cb_o", [PAIRS], i32, kind="ExternalOutput")

    with tile.TileContext(nc) as tc:
        with (
            tc.tile_pool(name="const", bufs=1) as cst,
            tc.tile_pool(name="setup", bufs=1) as stp,
            tc.tile_pool(name="big", bufs=_BIGB) as big,
            tc.tile_pool(name="head", bufs=_HDB) as hd,
            tc.tile_pool(name="ptr", bufs=_TRB, space="PSUM") as ptr,
            tc.tile_pool(name="pzz", bufs=1, space="PSUM") as pzz,
            tc.tile_pool(name="pmisc", bufs=1, space="PSUM") as pmisc,
            tc.tile_pool(name="pout", bufs=_POUTB, space="PSUM") as pout,
        ):
            # ---- critical path first: ht features -> zt matmul -> EZT ----
            ht_sb = stp.tile([128, 4, CF], f32)
            nc.sync.dma_start(ht_sb[:], d_ht.rearrange("(j p) c -> p j c", p=128))
            hl_sb = stp.tile([NL, CF], f32)
            nc.sync.dma_start(hl_sb[:], d_hl[:])
            wp = cst.tile([CF, _WPACKN], f32)
            nc.sync.dma_start(wp[:], d_wp[:])
            wrow = cst.tile([1, _WROWN], f32)
            nc.sync.dma_start(wrow[:], d_wr[:])
            ident = cst.tile([128, 128], f32)
            make_identity(nc, ident[:])
            ones2 = cst.tile([2, 128], bf16)
            nc.vector.memset(ones2[:], 1.0)
            b30hl = cst.tile([2, 512], bf16)
            nc.sync.dma_start(b30hl[:], d_bhl[:])

            w1a = wp[:, _W1A0:_W1A0 + 128]
            w1b = wp[:, _W1B0:_W1B0 + 128]
            w30 = wp[:, _W300:_W300 + 30]
            wat = wp[:, _WAT0:_WAT0 + 28]
            wbs = wp[:, _WBS0:_WBS0 + 6]
            wbd = wp[:, _WBD0:_WBD0 + 6]
            cvec = wp[:, _CV0:_CV0 + 1]
            b30rep = wrow[0:1, _B300:_B300 + 512]

            htT = stp.tile([CF, NT], f32)
            for j in range(4):
                p_tj = ptr.tile([128, 128], f32, tag="tr", name=f"ptj{j}")
                nc.tensor.transpose(p_tj[:, :], ht_sb[:, j, :], ident[:, :])
                nc.vector.tensor_copy(htT[:, j * 128:(j + 1) * 128], p_tj[:, :])
            hlT = stp.tile([CF, NL], f32)
            p_t1 = ptr.tile([128, 128], f32, tag="tr")
            nc.tensor.transpose(p_t1[:, 0:NL], hl_sb[:], ident[0:NL, 0:NL])
            nc.vector.tensor_copy(hlT[:], p_t1[:, 0:NL])

            # zl'' = W1a'^T @ hl^T + cvec ; EZL = exp(zl'') ; zl1 = zl''+1
            p_zl = pzz.tile([128, NT], f32, tag="zz")
            nc.tensor.matmul(p_zl[:, 0:NL], lhsT=w1a, rhs=hlT[:], start=True, stop=True)
            zl_sb = stp.tile([H, NL], f32)
            nc.scalar.activation(zl_sb[:], p_zl[:, 0:NL], AF.Identity, bias=cvec, scale=1.0)
            ezl = stp.tile([H, NL], f32)
            nc.scalar.activation(ezl[:], p_zl[:, 0:NL], AF.Exp, bias=cvec, scale=1.0)

            # ztp = W1b'^T @ ht^T ; EZT = exp(ztp)
            p_zt = pzz.tile([128, NT], f32, tag="zz")
            nc.tensor.matmul(p_zt[:], lhsT=w1b, rhs=htT[:], start=True, stop=True)
            ztp = stp.tile([H, NT], f32)
            nc.scalar.copy(ztp[:], p_zt[:])
            ezt = stp.tile([H, NT], f32)
            nc.scalar.activation(ezt[:], p_zt[:], AF.Exp)

            # ---------------- distances ----------------
            pl_sb = stp.tile([NL, 3], f32)
            nc.sync.dma_start(pl_sb[:], d_pl[:])
            pt_sb = stp.tile([128, 4, 3], f32)
            nc.sync.dma_start(pt_sb[:], d_pt.rearrange("(j p) d -> p j d", p=128))

            sqtmp = stp.tile([128, 4, 3], f32)
            plsq = stp.tile([NL, 1], f32)
            nc.vector.tensor_mul(sqtmp[0:NL, 0, :], pl_sb[:], pl_sb[:])
            nc.vector.tensor_reduce(
                out=plsq[:], in_=sqtmp[0:NL, 0:1, :], axis=AX.X, op=OP.add)
            sqt2 = stp.tile([128, 4, 3], f32)
            ptsq = stp.tile([128, 4], f32)
            nc.vector.tensor_mul(sqt2[:], pt_sb[:], pt_sb[:])
            nc.vector.tensor_reduce(out=ptsq[:], in_=sqt2[:], axis=AX.X, op=OP.add)

            # rhs_dist [5, 64] = [pl^T ; |pl|^2 ; 1]
            plq = stp.tile([NL, 5], f32)
            nc.vector.memset(plq[:, 4:5], 1.0)
            nc.vector.tensor_copy(plq[:, 0:3], pl_sb[:])
            nc.vector.tensor_copy(plq[:, 3:4], plsq[:])
            rhs_dist = stp.tile([5, NL], f32)
            p_tr = ptr.tile([128, 128], f32, tag="tr")
            nc.tensor.transpose(p_tr[0:5, 0:NL], plq[:], ident[0:NL, 0:NL])
            nc.vector.tensor_copy(rhs_dist[:], p_tr[0:5, 0:NL])

            # lhsT_dist [5, 512] = [-2*pt^T ; ones ; |pt|^2]
            ptq = stp.tile([128, 4, 5], f32)
            nc.vector.memset(ptq[:], 1.0)
            for j in range(4):
                nc.vector.tensor_scalar_mul(ptq[:, j, 0:3], pt_sb[:, j, :], -2.0)
            nc.vector.tensor_copy(
                bap(ptq, 4, [list(ptq.ap[0]), [5, 4], [1, 1]]), ptsq[:])
            lhsT_dist = stp.tile([5, NT], f32)
            for j in range(4):
                p_trj = ptr.tile([128, 128], f32, tag="tr", name=f"ptr{j}")
                nc.tensor.transpose(p_trj[0:5, :], ptq[:, j, :], ident[:, :])
                nc.vector.tensor_copy(lhsT_dist[:, j * 128:(j + 1) * 128], p_trj[0:5, :])

            # d2 fully in psum; dist = exp(0.5*ln(d2))
            dist_sb = stp.tile([128, 4, NL], f32)
            lntmp = stp.tile([128, 4, NL], f32)
            p_d = pmisc.tile([128, 4, NL], f32, tag="misc")
            for j in range(4):
                nc.tensor.matmul(
                    p_d[:, j, :], lhsT=lhsT_dist[:, j * 128:(j + 1) * 128],
                    rhs=rhs_dist[:], start=True, stop=True)
            nc.scalar.activation(lntmp[:], p_d[:], AF.Ln)
            nc.scalar.activation(dist_sb[:], lntmp[:], AF.Exp, scale=0.5)

            # ---------------- atom types ----------------
            bat = cst.tile([NL, 28], f32)
            nc.sync.dma_start(
                bat[:], bass.AP(d_wr[:].tensor, _BAT0, [[0, NL], [1, 28]]))
            bbd = cst.tile([EC, 6], f32)
            nc.sync.dma_start(
                bbd[:], bass.AP(d_wr[:].tensor, _BBD0, [[0, EC], [1, 6]]))

            p_at = pmisc.tile([128, 4, NL], f32, tag="misc")
            nc.tensor.matmul(p_at[0:NL, 0, 0:28], lhsT=hlT[:], rhs=wat, start=True, stop=True)
            atom_sb = stp.tile([NL, 28], f32)
            nc.vector.scalar_tensor_tensor(
                out=atom_sb[:], in0=p_at[0:NL, 0, 0:28], scalar=0.0,
                in1=bat[:], op0=OP.add, op1=OP.add)
            nc.sync.dma_start(d_atom[:], atom_sb[:])

            # ---------------- bond types (indirect row gather) ----------------
            es_sb = stp.tile([EC, 1], i32)
            nc.sync.dma_start(es_sb[:], d_es[:])
            ed_sb = stp.tile([EC, 1], i32)
            nc.sync.dma_start(ed_sb[:], d_ed[:])

            g_src = stp.tile([EC, CF], f32)
            nc.gpsimd.indirect_dma_start(
                out=g_src[:], out_offset=None, in_=d_hlx[:],
                in_offset=bass.IndirectOffsetOnAxis(ap=es_sb[:, 0:1], axis=0))
            g_dst = stp.tile([EC, CF], f32)
            nc.gpsimd.indirect_dma_start(
                out=g_dst[:], out_offset=None, in_=d_hlx[:],
                in_offset=bass.IndirectOffsetOnAxis(ap=ed_sb[:, 0:1], axis=0))
            gsT = stp.tile([CF, EC], f32)
            p_g1 = ptr.tile([128, 128], f32, tag="tr")
            nc.tensor.transpose(p_g1[:, :], g_src[:], ident[:, :])
            nc.vector.tensor_copy(gsT[:], p_g1[:, :])
            gdT = stp.tile([CF, EC], f32)
            p_g2 = ptr.tile([128, 128], f32, tag="tr")
            nc.tensor.transpose(p_g2[:, :], g_dst[:], ident[:, :])
            nc.vector.tensor_copy(gdT[:], p_g2[:, :])

            p_bd = pmisc.tile([128, 4, NL], f32, tag="misc")
            nc.tensor.matmul(p_bd[0:EC, 0, 0:6], lhsT=gsT[:], rhs=wbs, start=True, stop=False)
            nc.tensor.matmul(p_bd[0:EC, 0, 0:6], lhsT=gdT[:], rhs=wbd, start=False, stop=True)
            bond_sb = stp.tile([EC, 6], f32)
            nc.vector.scalar_tensor_tensor(
                out=bond_sb[:], in0=p_bd[0:EC, 0, 0:6], scalar=0.0,
                in1=bbd[:], op0=OP.add, op1=OP.add)
            nc.sync.dma_start(d_bond[:], bond_sb[:])

            # ---------------- C_batch ----------------
            cbv_sb = stp.tile([128, 1], i32)
            nc.sync.dma_start(
                cbv_sb[:], bass.AP(d_cbv[:].tensor, 0, [[0, 128], [1, 1]]))
            cb_sb = stp.tile([128, PAIRS // 128], i32)
            nc.vector.tensor_copy(
                cb_sb[:], bass.AP(cbv_sb.tensor, cbv_sb.offset,
                                  [list(cbv_sb.ap[0]), [0, PAIRS // 128]]))
            nc.sync.dma_start(d_cb.rearrange("(p n) -> p n", p=128), cb_sb[:])

            # ---------------- main pair loop ----------------
            out_v = d_out.rearrange("(s b p) c -> s p b c", s=NSUP, p=128)
            for g in range(NSUP):
                ez = big.tile([128, G, NT], f32, tag="ez", name=f"ez{g}")
                rz = big.tile([128, G, NT], f32, tag="rz", name=f"rz{g}")
                for li in range(G):
                    l = g * G + li
                    if _PAT[li % len(_PAT)] == "a":
                        nc.scalar.activation(
                            rz[:, li, :], ztp[:], AF.Relu,
                            bias=zl_sb[:, l:l + 1], scale=1.0)
                    else:
                        nc.gpsimd.tensor_scalar(
                            out=rz[:, li, :], in0=ztp[:],
                            scalar1=zl_sb[:, l:l + 1], scalar2=0.0,
                            op0=OP.add, op1=OP.max)
                    nc.vector.tensor_scalar(
                        out=ez[:, li, :], in0=ezt[:],
                        scalar1=ezl[:, l:l + 1], scalar2=1.0,
                        op0=OP.mult, op1=OP.min)

                # psum = b30' + E@W30 + R@W30, 32 blocks of 32 cols
                po = pout.tile([128, POUT_BANKS, 512], f32, tag="out", name=f"po{g}")
                for bk in range(POUT_BANKS):
                    nc.tensor.matmul(po[:, bk, :], lhsT=ones2[:], rhs=b30hl[:],
                                     start=True, stop=False, skip_group_check=True)
                for li in range(G):
                    for c4 in range(4):
                        blk = li * 4 + c4
                        bank, off = divmod(blk * 32, 512)
                        sl = po[:, bank, off:off + 30]
                        nc.tensor.matmul(sl, lhsT=ez[:, li, c4 * 128:(c4 + 1) * 128],
                                         rhs=w30, start=False, stop=False,
                                         skip_group_check=True)
                        nc.tensor.matmul(sl, lhsT=rz[:, li, c4 * 128:(c4 + 1) * 128],
                                         rhs=w30, start=False, stop=(blk == BLK - 1),
                                         skip_group_check=True)

                blocks = po.rearrange("p a b -> p (a b)").rearrange(
                    "p (blk c) -> p blk c", c=32)          # [128, 32, 32]

                piC = hd.tile([128, BLK, 10], f32, tag="piC", name=f"piC{g}")
                nc.scalar.activation(piC[:], blocks[:, :, 0:10], AF.Exp)
                eC = hd.tile([128, BLK, 20], f32, tag="eC", name=f"eC{g}")
                nc.scalar.activation(eC[:], blocks[:, :, 10:30], AF.Exp)
                rC = hd.tile([128, BLK, 20], f32, tag="rC", name=f"rC{g}")
                nc.scalar.activation(rC[:], blocks[:, :, 10:30], AF.Relu)

                ssum = hd.tile([128, BLK], f32, tag="ssum", name=f"ss{g}")
                nc.vector.tensor_reduce(out=ssum[:], in_=piC[:], axis=AX.X, op=OP.add)
                rcp = hd.tile([128, BLK], f32, tag="rcp", name=f"rcp{g}")
                nc.vector.reciprocal(rcp[:], ssum[:])

                stg = hd.tile([128, BLK, 32], f32, tag="stg", name=f"stg{g}")
                nc.vector.memset(stg[:, :, 31:32], 0.0)
                tt_eng = nc.gpsimd if _TT_GPS else nc.vector
                # pi = e * (1/sum)
                tt_eng.tensor_tensor(
                    out=stg[:, :, 0:10], in0=piC[:],
                    in1=bass.AP(rcp.tensor, rcp.offset, list(rcp.ap) + [[0, 10]]),
                    op=OP.mult)
                # mu = min(e,1) + relu   [= elu+1]
                nc.vector.scalar_tensor_tensor(
                    out=stg[:, :, 20:30], in0=eC[:, :, 10:20], scalar=1.0,
                    in1=rC[:, :, 10:20], op0=OP.min, op1=OP.add)
                # sigma = min(e+0.1, 1.1) + relu   [= elu+1.1]
                tsg = hd.tile([128, BLK, 10], f32, tag="tsg", name=f"tsg{g}")
                nc.gpsimd.tensor_scalar(
                    out=tsg[:], in0=eC[:, :, 0:10],
                    scalar1=0.1, scalar2=1.1, op0=OP.add, op1=OP.min)
                tt_eng.tensor_tensor(
                    out=stg[:, :, 10:20], in0=tsg[:], in1=rC[:, :, 0:10], op=OP.add)
                # dist scatter into column 30 (block = 4*l' + j)
                stg_d = bass.AP(stg.tensor, stg.offset + 30,
                                [list(stg.ap[0]), [128, G], [32, 4], [1, 1]])
                dst_d = bass.AP(dist_sb.tensor, dist_sb.offset + g * G,
                                [list(dist_sb.ap[0]), [1, G], [NL, 4], [1, 1]])
                nc.gpsimd.tensor_copy(out=stg_d, in_=dst_d)

                nc.sync.dma_start(out_v[g], stg[:])

    nc.compile()
    return nc


def _get_program():
    global _PROGRAM
    if _PROGRAM is None:
        _PROGRAM = build_program()
    return _PROGRAM


def prep_in_maps(inputs):
    """Fold BN into weights, pack small weights, shard per core."""
    h_l_x = np.ascontiguousarray(np.asarray(inputs["h_l_x"], np.float32))
    h_t_x = np.asarray(inputs["h_t_x"], np.float32)
    h_l_pos = np.asarray(inputs["h_l_pos"], np.float32)
    h_t_pos = np.asarray(inputs["h_t_pos"], np.float32)
    eidx = np.asarray(inputs["edge_index"])
    W1 = np.asarray(inputs["W1"], np.float32)
    b1 = np.asarray(inputs["b1"], np.float32)
    gamma = np.asarray(inputs["gamma"], np.float32)
    beta = np.asarray(inputs["beta"], np.float32)
    rmean = np.asarray(inputs["rmean"], np.float32)
    rvar = np.asarray(inputs["rvar"], np.float32)

    s = gamma.astype(np.float64) / np.sqrt(rvar.astype(np.float64) + BN_EPS)
    cvec = ((b1.astype(np.float64) - rmean) * s + beta).astype(np.float32)
    w1a = (W1[:CF].astype(np.float64) * s[None, :]).astype(np.float32)
    w1b = (W1[CF:].astype(np.float64) * s[None, :]).astype(np.float32)

    W30 = np.concatenate(
        [np.asarray(inputs["Wpi"], np.float32),
         np.asarray(inputs["Wsig"], np.float32),
         np.asarray(inputs["Wmu"], np.float32)], axis=1)
    b30 = np.concatenate(
        [np.asarray(inputs["bpi"], np.float32),
         np.asarray(inputs["bsig"], np.float32),
         np.asarray(inputs["bmu"], np.float32)])
    b30p = (b30.astype(np.float64) - W30.astype(np.float64).sum(0)).astype(np.float32)
    b30rep = np.tile(np.concatenate([b30p, np.zeros(2, np.float32)]), 16)

    Wbd = np.asarray(inputs["Wbd"], np.float32)
    wpack = np.zeros((CF, _WPACKN), np.float32)
    wpack[:, _W1A0:_W1A0 + 128] = w1a
    wpack[:, _W1B0:_W1B0 + 128] = w1b
    wpack[:, _W300:_W300 + 30] = W30
    wpack[:, _WAT0:_WAT0 + 28] = np.asarray(inputs["Wat"], np.float32)
    wpack[:, _WBS0:_WBS0 + 6] = Wbd[:CF]
    wpack[:, _WBD0:_WBD0 + 6] = Wbd[CF:]
    wpack[:, _CV0] = cvec

    import ml_dtypes
    b30_hi = b30rep.astype(ml_dtypes.bfloat16)
    b30_lo = (b30rep.astype(np.float64) - b30_hi.astype(np.float64)).astype(
        np.float32).astype(ml_dtypes.bfloat16)
    b30hl = np.stack([np.asarray(b30_hi), np.asarray(b30_lo)])  # [2, 512] bf16

    wrow = np.zeros((1, _WROWN), np.float32)
    wrow[0, _B300:_B300 + 512] = b30rep
    wrow[0, _BAT0:_BAT0 + 28] = np.asarray(inputs["bat"], np.float32)
    wrow[0, _BBD0:_BBD0 + 6] = np.asarray(inputs["bbd"], np.float32)

    shared = dict(hlx=h_l_x, wpack=wpack, wrow=wrow, b30hl=b30hl)

    in_maps = []
    for c in range(NCORES):
        m = dict(shared)
        m["hl"] = np.ascontiguousarray(h_l_x[c * NL:(c + 1) * NL])
        m["ht"] = np.ascontiguousarray(h_t_x[c * NT:(c + 1) * NT])
        m["pl"] = np.ascontiguousarray(h_l_pos[c * NL:(c + 1) * NL])
        m["pt"] = np.ascontiguousarray(h_t_pos[c * NT:(c + 1) * NT])
        m["esrc"] = np.ascontiguousarray(
            eidx[0, c * EC:(c + 1) * EC].astype(np.int32).reshape(EC, 1))
        m["edst"] = np.ascontiguousarray(
            eidx[1, c * EC:(c + 1) * EC].astype(np.int32).reshape(EC, 1))
        m["cbval"] = np.full((1, 1), c, np.int32)
        in_maps.append(m)
    return in_maps


def assemble(results):
    """results: list of 8 dicts keyed by output tensor name -> full outputs."""
    out_all = np.stack([np.asarray(r["out_all"]) for r in results])   # [8,32768,32]
    pi = np.ascontiguousarray(out_all[:, :, 0:10]).reshape(B * PAIRS, 10)
    sigma = np.ascontiguousarray(out_all[:, :, 10:20]).reshape(B * PAIRS, 10)
    mu = np.ascontiguousarray(out_all[:, :, 20:30]).reshape(B * PAIRS, 10)
    dist = np.ascontiguousarray(out_all[:, :, 30:31]).reshape(B * PAIRS, 1)
    atom = np.concatenate([np.asarray(r["atom_o"]) for r in results], 0)
    bond = np.concatenate([np.asarray(r["bond_o"]) for r in results], 0)
    cb = np.concatenate(
        [np.asarray(r["cb_o"]).reshape(-1) for r in results], 0).astype(np.int32)
    return (pi, sigma, mu, dist, atom, bond, cb)


def kernel(**inputs):
    from concourse.bass_utils import run_bass_kernel_spmd

    nc = _get_program()
    in_maps = prep_in_maps(inputs)
    res = run_bass_kernel_spmd(nc, in_maps, core_ids=list(range(NCORES)))
    return assemble(res.results)


if __name__ == "__main__":
    prog = build_program()
    n = sum(len(bb.instructions) for bb in prog.main_func.blocks)
    print(f"program built ok: {n} instructions")
